# revision 2
# baseline (speedup 1.0000x reference)
"""Causal multi-head attention (B=4, S=2048, E=1024, H=16) on 8 trn2 NeuronCores.

Sharding: core c = (batch b = c//2, head-group g = c%2). Each core computes
attention for one batch element and 8 of the 16 heads, plus the partial
out-projection restricted to its heads' columns. Host sums the two partial
outputs per batch element and adds the out-projection bias.

v2: bf16 data path (PSUM accumulation stays f32), host-packed DMA layouts.
  - x is host-packed as xq[w, p, e, t] (w = 512-token window, p = E-row%128,
    e = E-row//128, t = token-in-window) so each per-window DMA is 128
    descriptors of 8KB instead of 1024x1KB. One DMA per window on the sync
    queue; nothing else runs on sync, so x prefetch is never head-of-line
    blocked.
  - out is written per window as one [128, 4, 1024] bf16 tile -> one DMA of
    128 descriptors on the gpsimd queue (host transposes back and upcasts).
  - everything on-device flows in "transposed" space so no on-device
    transposes are needed:
      qT, kT  [ch, s]   from  W_chunk @ x.T        (lhsT = W.T tiles, rhs = x.T)
      v       [s, ch]   from  x @ Wv.T             (lhsT = x.T tiles, rhs = Wv.T)
      scoresT [k, q]    from  lhsT = kT, rhs = qT  (per head, K = 64)
      ctxT    [d, q]    from  lhsT = v (+ones col), rhs = exp(scoresT)
      outP    [s, o]    from  lhsT = ctxT, rhs = Wo.T
  - softmax without max subtraction (scores are bounded ~|2|); normalizer
    from a ones-column appended to v; causal mask is a multiplicative 0/1
    triangular 128x128 mask on the exp'd diagonal blocks. bf16 matmuls have
    no N>=256 full-rate restriction, so diagonal blocks are trimmed to the
    exact causal width (w0 = t*128).
"""

import sys

sys.path.insert(0, "/opt/trn_rl_repo")

import numpy as np
import ml_dtypes

import concourse.bass as bass  # noqa: F401  (registers engine classes)
import concourse.mybir as mybir
import concourse.tile as tile
from concourse import bacc
from concourse.bass_utils import run_bass_kernel_spmd

F32 = mybir.dt.float32
BF16 = mybir.dt.bfloat16
AF = mybir.ActivationFunctionType

B, S, E = 4, 2048, 1024
H, HD = 16, 64
GH = 8                 # heads handled per core
GC = GH * HD           # 512 channels per head-group
P = 128
NCORES = 8
NJ_ALL = S // P        # 16 k-blocks of 128
QB = S // 512          # 4 q-windows of 512

_program = {}


def _emit(tc, nc, xq, wqkT, wvT, woT, bqk, bv, out, bench_iters=0,
          has_bias=True):
    ctxmgr = []

    def pool(**kw):
        p = tc.tile_pool(**kw)
        ctxmgr.append(p)
        return p.__enter__()

    const = pool(name="const", bufs=1)
    kvp = pool(name="kv", bufs=1)
    xp = pool(name="xs", bufs=2)
    qp = pool(name="qt", bufs=2)
    cxp = pool(name="ctx", bufs=2)
    ep = pool(name="expt", bufs=3)
    osb = pool(name="osb", bufs=2)
    bp = pool(name="bcast", bufs=2)
    ps_s = pool(name="ps_s", bufs=2, space="PSUM")
    ps_m = pool(name="ps_m", bufs=4, space="PSUM")

    # ---- constants ----
    # DMA order matters at startup: the first qkT matmuls need wqk + the
    # first x window; wo is only needed ~100us in (first out-projection),
    # so it is emitted last.
    bqk_sb = const.tile([P, 8], F32)
    nc.sync.dma_start(bqk_sb[:], bqk.rearrange("c p -> p c"))
    bv_sb = const.tile([P, 4], F32)
    nc.sync.dma_start(bv_sb[:], bv.rearrange("c p -> p c"))
    # Per-chunk DMAs so the first qkT matmul (which only reads chunk e=0)
    # can start as soon as its chunk lands, not after the whole 2MB.
    wqk_sb = const.tile([P, 8, 2 * GC], BF16)      # [p, e, ch]: W row e*128+p
    wqk_r = wqkT.rearrange("(eo p) c -> p eo c", p=P)
    for e in range(8):
        eng = nc.sync if e % 2 == 0 else nc.gpsimd
        eng.dma_start(wqk_sb[:, e, :], wqk_r[:, e, :])
    wv_sb = const.tile([P, 8, GC], BF16)
    wv_r = wvT.rearrange("(eo p) c -> p eo c", p=P)
    for e in range(8):
        eng = nc.gpsimd if e % 2 == 0 else nc.sync
        eng.dma_start(wv_sb[:, e, :], wv_r[:, e, :])
    wo_sb = const.tile([P, 4, E], BF16)
    wo_r = woT.rearrange("(co p) o -> p co o", p=P)
    for co in range(4):
        nc.gpsimd.dma_start(wo_sb[:, co, :], wo_r[:, co, :])

    # Causal 128x128 block mask: mtri[p, q'] = 1 if q' >= p else 0.
    # Built in f32 (memset/affine_select don't take bf16), then cast.
    tmpp = tc.tile_pool(name="tmpf", bufs=1)
    tmp = tmpp.__enter__()
    mtri_f = tmp.tile([P, P], F32)
    nc.gpsimd.memset(mtri_f[:], 1.0)
    nc.gpsimd.affine_select(
        out=mtri_f[:],
        in_=mtri_f[:],
        compare_op=mybir.AluOpType.is_ge,
        fill=0.0,
        base=0,
        pattern=[[1, P]],        # + q'
        channel_multiplier=-1,   # - p   => keep where q' - p >= 0
    )
    mtri = const.tile([P, P], BF16)
    nc.vector.tensor_copy(mtri[:], mtri_f[:])

    # ---- persistent K^T and V ----
    kT_sb = kvp.tile([P, 4, S], BF16)            # [p, c, s]; ch = c*128+p
    v_sb = kvp.tile([P, NJ_ALL, GH, HD + 1], BF16)  # [s%128, j, h, d(+ones)]
    ones_f = tmp.tile([P, NJ_ALL * GH], F32)
    nc.vector.memset(ones_f[:], 1.0)
    nc.vector.tensor_copy(
        v_sb[:, :, :, HD],
        ones_f[:].rearrange("p (j h) -> p j h", j=NJ_ALL),
    )
    tmpp.__exit__(None, None, None)

    if not has_bias:
        bqk_sb = bv_sb = None
    import contextlib
    if bench_iters:
        # large body (>256 insts/engine): arm branch prefetch so the
        # back-edge doesn't stall on an IRAM refetch every iteration
        loop_cm = tc.For_i(0, bench_iters, 1,
                           hint_engines=(mybir.EngineType.PE,
                                         mybir.EngineType.DVE,
                                         mybir.EngineType.Activation,
                                         mybir.EngineType.Pool,
                                         mybir.EngineType.SP))
    else:
        loop_cm = contextlib.nullcontext()
    with loop_cm:
        _emit_body(tc, nc, xq, out, wqk_sb, wv_sb, wo_sb, bqk_sb, bv_sb,
                   mtri, kT_sb, v_sb, qp, xp, cxp, ep, osb, bp, ps_s, ps_m)

    for p in reversed(ctxmgr):
        p.__exit__(None, None, None)


def _emit_body(tc, nc, xq, out, wqk_sb, wv_sb, wo_sb, bqk_sb, bv_sb,
               mtri, kT_sb, v_sb, qp, xp, cxp, ep, osb, bp, ps_s, ps_m):
    """Software-pipelined emission: attention(qb) is the ACT-paced backbone;
    PE-only work — qkv(qb+1) chains and outproj(qb-1) chains — is spliced
    between individual j-iterations so the in-order PE stream always has
    independent matmuls to chew on while it waits for exp results."""
    # x for the whole iteration: one DMA per 512-token window, 128
    # descriptors each (host-packed layout), all on the sync queue.
    xw = xp.tile([P, QB, 8, 512], BF16, tag="xw")
    for w in range(QB):
        nc.sync.dma_start(xw[:, w], xq[w])

    def new_qT(qb):
        return qp.tile([P, 4, 512], BF16, tag="qT", name=f"qT{qb % 2}")

    def run_chains(gen):
        for chain in gen:
            chain()

    qT_cur = new_qT(0)
    run_chains(_qkv_chains(tc, nc, xw, wqk_sb, wv_sb, bqk_sb,
                           kT_sb, v_sb, ps_m, 0, qT_cur))
    ctx_prev = None
    for qb in range(QB):
        qT_next = new_qT(qb + 1) if qb + 1 < QB else None
        fillers = []
        if qT_next is not None:
            fillers.append(_qkv_chains(tc, nc, xw, wqk_sb, wv_sb, bqk_sb,
                                       kT_sb, v_sb, ps_m, qb + 1, qT_next))
        if ctx_prev is not None:
            fillers.append(_outproj_chains(tc, nc, out, wo_sb, ctx_prev,
                                           osb, ps_m, qb - 1))
        ctx_prev = _attn(tc, nc, mtri, kT_sb, v_sb, bv_sb, qT_cur,
                         cxp, ep, bp, ps_s, ps_m, qb,
                         fillers=_roundrobin(fillers))
        qT_cur = qT_next
    run_chains(_outproj_chains(tc, nc, out, wo_sb, ctx_prev, osb, ps_m,
                               QB - 1))


def _roundrobin(gens):
    gens = list(gens)
    while gens:
        g = gens.pop(0)
        try:
            yield next(g)
            gens.append(g)
        except StopIteration:
            pass


def _qkv_chains(tc, nc, xw, wqk_sb, wv_sb, bqk_sb, kT_sb, v_sb,
                ps_m, qb, qT):
    """Yield one callable per accumulation chain (8 matmuls + a drain op)."""
    def qk_chain(cb):
        pq = ps_m.tile([P, 512], F32, tag="m")
        for e in range(8):
            nc.tensor.matmul(
                pq[:],
                wqk_sb[:, e, cb * P:(cb + 1) * P],
                xw[:, qb, e, :],
                start=(e == 0), stop=(e == 7),
            )
        if cb < 4:
            dest = qT[:, cb, :]
        else:
            dest = kT_sb[:, cb - 4, qb * 512:(qb + 1) * 512]
        if bqk_sb is not None:
            nc.vector.tensor_scalar_add(dest, pq[:], bqk_sb[:, cb:cb + 1])
        else:
            nc.vector.tensor_copy(dest, pq[:])

    def v_chain(jj):
        j = 4 * qb + jj
        pv = ps_m.tile([P, 512], F32, tag="m")
        for e in range(8):
            nc.tensor.matmul(
                pv[:],
                xw[:, qb, e, jj * P:(jj + 1) * P],
                wv_sb[:, e, :],
                start=(e == 0), stop=(e == 7),
            )
        nc.vector.tensor_copy(
            v_sb[:, j, :, 0:HD],
            pv[:].rearrange("p (h d) -> p h d", h=GH),
        )

    # k and v chains first: the next qb's attention needs kT/v before qT
    for cb in (4, 5, 6, 7):
        yield (lambda cb=cb: qk_chain(cb))
    for jj in range(4):
        yield (lambda jj=jj: v_chain(jj))
    for cb in (0, 1, 2, 3):
        yield (lambda cb=cb: qk_chain(cb))


def _emit_pv(nc, pv2, v_sb, c, hp, j, w0, ex, nj):
    nc.tensor.matmul(
        pv2[hp][0:HD + 1, w0:512],
        v_sb[:, j, 2 * c + hp, :],
        ex[:, hp, w0:512],
        start=(j == 0), stop=(j == nj - 1),
    )


def _outproj_chains(tc, nc, out, wo_sb, ctxT, osb, ps_m, qb):
    ot = osb.tile([P, 4, E], BF16, tag="ot")
    for sb_i in range(4):
        for ob in range(2):
            def chain(sb_i=sb_i, ob=ob):
                po = ps_m.tile([P, 512], F32, tag="m")
                for cc in range(4):
                    nc.tensor.matmul(
                        po[:],
                        ctxT[:, cc, sb_i * P:(sb_i + 1) * P],
                        wo_sb[:, cc, ob * 512:(ob + 1) * 512],
                        start=(cc == 0), stop=(cc == 3),
                    )
                nc.vector.tensor_copy(
                    ot[:, sb_i, ob * 512:(ob + 1) * 512], po[:])
            yield chain

    def store():
        nc.gpsimd.dma_start(out[qb], ot[:])
    yield store


def _attn(tc, nc, mtri, kT_sb, v_sb, bv_sb, qT, cxp, ep, bp, ps_s, ps_m,
          qb, fillers=None):
    # ---- attention for this q-window ----
    # Heads 2c (SBUF partitions 0-63) and 2c+1 (64-127) are processed
    # together: their score matmuls land on PE row-groups (0,0)/(64,0)
    # and overlap in the array.
    ctxT = cxp.tile([P, 4, 512], BF16)       # [p, c, q]; ch = c*128+p
    nj = 4 * (qb + 1)
    fillers = iter(fillers) if fillers is not None else iter(())
    done = False
    n_iters = 4 * nj
    acc = 0.0
    per_iter = 21.0 / n_iters   # ~21 filler chains spread over the window

    def emit_fillers(force_all=False):
        nonlocal acc, done
        if done:
            return
        acc += per_iter
        while (acc >= 1.0 or force_all) and not done:
            acc -= 1.0
            try:
                next(fillers)()
            except StopIteration:
                done = True

    for c in range(4):
        pv2 = [ps_m.tile([P, 512], F32, tag="m", name=f"pv{hp}")
               for hp in range(2)]
        pend = []   # software-pipeline: PV trails scores by one (j, hp)
        for j in range(nj):
            t = j - 4 * qb
            # Diagonal blocks only need q-columns >= t*128 (causality).
            w0 = 0 if t < 0 else t * P
            sp = ps_s.tile([P, 2, 512], F32)
            ex = ep.tile([P, 2, 512], BF16)
            for hp in range(2):
                p0 = 64 * hp
                nc.tensor.matmul(
                    sp[:, hp, w0:512],
                    kT_sb[p0:p0 + 64, c, j * P:(j + 1) * P],
                    qT[p0:p0 + 64, c, w0:512],
                    start=True, stop=True,
                )
            for hp in range(2):
                nc.scalar.activation(ex[:, hp, w0:512], sp[:, hp, w0:512],
                                     AF.Exp)
                if t >= 0:
                    # mask multiply over the diagonal 128x128 band
                    nc.vector.tensor_mul(
                        ex[:, hp, w0:w0 + P],
                        ex[:, hp, w0:w0 + P],
                        mtri[:],
                    )
                if pend:
                    _emit_pv(nc, pv2, v_sb, c, *pend.pop(0), nj)
                pend.append((hp, j, w0, ex))
            emit_fillers()
        while pend:
            _emit_pv(nc, pv2, v_sb, c, *pend.pop(0), nj)
        # normalize: ctxT = pv[0:64] / pv[64] (+ v bias)
        for hp in range(2):
            p0 = 64 * hp
            pv_ps = pv2[hp]
            bc = bp.tile([64, 512], F32)
            nc.vector.reciprocal(bc[0:1, :], pv_ps[HD:HD + 1, :])
            nc.gpsimd.partition_broadcast(bc[:], bc[0:1, :])
            nc.vector.tensor_mul(ctxT[p0:p0 + 64, c, :], pv_ps[0:HD, :], bc[:])
            if bv_sb is not None:
                nc.vector.tensor_scalar_add(
                    ctxT[p0:p0 + 64, c, :],
                    ctxT[p0:p0 + 64, c, :],
                    bv_sb[p0:p0 + 64, c:c + 1],
                )
    emit_fillers(force_all=True)
    return ctxT


def _build_program(bench_iters=0, has_bias=True):
    nc = bacc.Bacc("TRN2", target_bir_lowering=False, debug=False,
                   num_devices=NCORES)
    xq = nc.dram_tensor("xq", [QB, P, 8, 512], BF16, kind="ExternalInput").ap()
    wqkT = nc.dram_tensor("wqkT", [E, 2 * GC], BF16, kind="ExternalInput").ap()
    wvT = nc.dram_tensor("wvT", [E, GC], BF16, kind="ExternalInput").ap()
    woT = nc.dram_tensor("woT", [GC, E], BF16, kind="ExternalInput").ap()
    bqk = nc.dram_tensor("bqk", [8, P], F32, kind="ExternalInput").ap()
    bv = nc.dram_tensor("bv", [4, P], F32, kind="ExternalInput").ap()
    # out[w, p, sb, o]: token = w*512 + sb*128 + p
    out = nc.dram_tensor("o", [QB, P, 4, E], BF16, kind="ExternalOutput").ap()
    with tile.TileContext(nc) as tc:
        _emit(tc, nc, xq, wqkT, wvT, woT, bqk, bv, out,
              bench_iters=bench_iters, has_bias=has_bias)
    nc.compile()
    return nc


def _get_program(has_bias=True):
    if has_bias not in _program:
        _program[has_bias] = _build_program(has_bias=has_bias)
    return _program[has_bias]


def _make_in_maps(x, in_proj_w, in_proj_b, out_proj_w):
    scale = np.float32(1.0 / np.sqrt(HD))
    bf16 = ml_dtypes.bfloat16
    in_maps = []
    for c in range(NCORES):
        b, g = divmod(c, 2)
        lo, hi = g * GC, (g + 1) * GC
        wq = in_proj_w[lo:hi, :]
        wk = in_proj_w[E + lo:E + hi, :]
        wv = in_proj_w[2 * E + lo:2 * E + hi, :]
        wqkT = np.concatenate([wq.T * scale, wk.T], axis=1)
        wvT = np.ascontiguousarray(wv.T)
        woT = np.ascontiguousarray(out_proj_w[:, lo:hi].T)
        bqk = np.concatenate([in_proj_b[lo:hi] * scale,
                              in_proj_b[E + lo:E + hi]]).reshape(8, P)
        bvv = in_proj_b[2 * E + lo:2 * E + hi].reshape(4, P)
        # xq[w, p, e, t] = x[b, w*512+t, e*128+p]
        xq = np.ascontiguousarray(
            x[b].reshape(QB, 512, 8, P).transpose(0, 3, 2, 1))
        in_maps.append({
            "xq": xq.astype(bf16),
            "wqkT": np.ascontiguousarray(wqkT).astype(bf16),
            "wvT": wvT.astype(bf16),
            "woT": woT.astype(bf16),
            "bqk": np.ascontiguousarray(bqk, dtype=np.float32),
            "bv": np.ascontiguousarray(bvv, dtype=np.float32),
        })
    return in_maps


def _combine(results, out_proj_b):
    out = np.empty((B, S, E), dtype=np.float32)
    for b in range(B):
        # o[w, p, sb, o] -> [w, sb, p, o] -> [S, E]
        o0 = results[2 * b]["o"].astype(np.float32)
        o1 = results[2 * b + 1]["o"].astype(np.float32)
        out[b] = (o0 + o1).transpose(0, 2, 1, 3).reshape(S, E)
    out += np.asarray(out_proj_b, dtype=np.float32)[None, None, :]
    return out


def kernel(x, in_proj_w, in_proj_b, out_proj_w, out_proj_b, _trace=False):
    x = np.asarray(x, dtype=np.float32)
    in_proj_w = np.asarray(in_proj_w, dtype=np.float32)
    in_proj_b = np.asarray(in_proj_b, dtype=np.float32)
    out_proj_w = np.asarray(out_proj_w, dtype=np.float32)
    out_proj_b = np.asarray(out_proj_b, dtype=np.float32)
    assert x.shape == (B, S, E), x.shape

    has_bias = bool(np.any(in_proj_b))
    nc = _get_program(has_bias=has_bias)
    in_maps = _make_in_maps(x, in_proj_w, in_proj_b, out_proj_w)
    res = run_bass_kernel_spmd(nc, in_maps, core_ids=list(range(NCORES)),
                               trace=_trace)
    out = _combine(res.results, out_proj_b)
    if _trace:
        return out, res
    return out
